# revision 24
# baseline (speedup 1.0000x reference)
"""LoFTR-style LocalFeatureTransformer (linear attention) on 8 Trainium2 cores.

Sharding: core c <-> (batch b = c//2, sequence half h = c%2). Each core holds
channel-major [256, 2400] shards of BOTH features, SBUF-resident across all
8 layers. Linear attention's KV state ([hd, hv] plus a K-sum column) is
partial-summed over the local half-sequence and AllReduced (fp16) across the
2-core pair that shares a batch.

v3 structure (per feature update U_k = one encoder layer applied to one
feature; 16 updates total):
  A: Q proj + phi(q) for all 5 token tiles (independent of the AllReduce)
  [self-layer next: phase1(U_{k+1}) + its AllReduce issued HERE, so the
   collective flies under stages B..E]
  fk: unpack the AllReduced KV state
  B: Zden, reciprocal via Exp(-Ln), msg = KV q / Z      (stage loop, 5 tiles)
  C: centered merge -> LN1 (Square/var/Rsqrt/scale)
  D: W1 + bias + relu
  E: centered W2 -> LN2 -> residual into x
  [cross-layer next: phase1(U_{k+1}) + AllReduce issued here]
Stage loops let tile t+1's matmuls overlap tile t's DVE/ACT work (the v2
per-tile fused chain serialized on the PSUM ring).

Key tricks:
- Wm and W2 are CENTERED on the host (W - rowmean(W)): the merge/W2 matmuls
  directly produce mean-subtracted LayerNorm inputs; variance is mean(y^2).
- ln1's affine is folded into W1/bias1 on the host.
- ACT tables: phi/Z use the natural_log_exp table (1/x = Exp(-Ln(x))),
  LN rsqrt is native Rsqrt (one op; validated by the fp32 reference check),
  Copy/Relu/Square live in EVERY table. ~2 table loads per update.
- phi(x) = elu(x)+1 = exp(min(x,0)) + max(x,0): DVE min, ACT exp, fused DVE
  scalar_tensor_tensor. K-phi processes PAIRED 128-token blocks ([128,512]
  PSUM tiles) to amortize per-op overhead; V copy is paired the same way,
  with Ksum accumulated by N=1 matmuls against a constant ones column.
- Zden per (token, head) is computed replicated across each head's 32
  channels with a block-diagonal column-replicated Ksum matrix (1/64 scale
  keeps fp16 KV in range; cancels exactly in Z).
- reference computes msg = Q (K^T (v/S)) Z * S; the /S and *S cancel.
- All matmuls fp16, fp32 accumulation in PSUM.
"""

import numpy as np

import concourse.bass as bass
import concourse.mybir as mybir
import concourse.tile as tile
import bass_rust

N_CORES = 8
B, L, C = 4, 4800, 256
NHEAD, HDIM = 8, 32
R = L // 2              # tokens per core per feature: 2400
TT = 480                # channel-major token tile (moving operand, <=512)
NTT = R // TT           # 5
# paired 128-token blocks for phase1: (start, n0, n1) with n1 == 0 for the tail
PAIR_BLOCKS = [(i * 256, 128, min(128, R - i * 256 - 128)) for i in range(R // 256)]
if R % 256:
    PAIR_BLOCKS.append((R - R % 256, R % 256, 0))
EPS_LN = 1e-5
GROUPS = [[0, 1], [2, 3], [4, 5], [6, 7]]

F32 = mybir.dt.float32
F16 = mybir.dt.float16

_ws_ctr = [0]


def split_multi_waits(nc, max_waits=1):
    """This walrus build accepts only ONE sync-wait per engine instruction.
    After TileContext exit (waits final), move excess waits onto
    EventSemaphore instructions inserted just before the owner."""
    n_split = 0
    for bb in nc.main_func.blocks:
        new_list = []
        for inst in bb.instructions:
            si = inst.sync_info
            waits = list(si.on_wait) if si is not None else []
            if len(waits) > max_waits:
                keep, extra = waits[:max_waits], waits[max_waits:]
                for w in extra:
                    _ws_ctr[0] += 1
                    ev = mybir.InstEventSemaphore(name=f"I-waitsplit-{_ws_ctr[0]}")
                    ev.engine = inst.engine
                    ev.sync_info = bass_rust.SyncInfo(on_wait=[w], on_update=[])
                    nc.register_instruction(ev)
                    new_list.append(ev)
                inst.sync_info = bass_rust.SyncInfo(
                    on_wait=keep, on_update=list(si.on_update)
                )
                n_split += 1
            new_list.append(inst)
        bb.instructions = new_list
    return n_split


def _act_raw(nc, out, in_, func, bias=0.0, scale=1.0):
    """nc.scalar.activation without the Reciprocal/Rsqrt ValueError guard.
    End-to-end error is validated against the fp32 reference."""
    eng = nc.scalar
    inputs = [eng.lower_ap(in_)]
    if not isinstance(bias, float):
        bias_arg = eng.lower_ap(bias)
    else:
        bias_arg = mybir.ImmediateValue(dtype=mybir.dt.float32, value=bias)
    inputs.append(bias_arg)
    inputs.append(mybir.ImmediateValue(dtype=mybir.dt.float32, value=scale))
    inputs.append(mybir.ImmediateValue(dtype=mybir.dt.float32, value=0.0))
    return eng.add_instruction(
        mybir.InstActivation(
            name=nc.get_next_instruction_name(),
            func=func,
            ins=inputs,
            outs=[eng.lower_ap(out)],
        )
    )


def build(n_layers=8):
    nc = bass.Bass("TRN2", target_bir_lowering=False, debug=False,
                   num_devices=N_CORES)
    AF = mybir.ActivationFunctionType
    OP = mybir.AluOpType

    xin = [nc.declare_dram_parameter(f"xT{f}", [C, R], F16, isOutput=False)
           for f in (0, 1)]
    wq_d = nc.declare_dram_parameter("Wq", [n_layers, C, C], F16, isOutput=False)
    wk_d = nc.declare_dram_parameter("Wk", [n_layers, C, C], F16, isOutput=False)
    wv_d = nc.declare_dram_parameter("Wv", [n_layers, C, C], F16, isOutput=False)
    wm_d = nc.declare_dram_parameter("Wm", [n_layers, C, C], F16, isOutput=False)
    w1_d = nc.declare_dram_parameter("W1", [n_layers, 2 * C, 2 * C], F16, isOutput=False)
    w2_d = nc.declare_dram_parameter("W2", [n_layers, 2 * C, C], F16, isOutput=False)
    b1_d = nc.declare_dram_parameter("b1p", [n_layers, 128, 4], F32, isOutput=False)
    l2w_d = nc.declare_dram_parameter("l2wp", [n_layers, 128, 2], F32, isOutput=False)
    l2b_d = nc.declare_dram_parameter("l2bp", [n_layers, 128, 2], F32, isOutput=False)
    mask_d = nc.declare_dram_parameter("blockmask", [128, 128], F16, isOutput=False)
    ones_d = nc.declare_dram_parameter("onesC", [128, 128], F16, isOutput=False)
    yout = [nc.declare_dram_parameter(f"yT{f}", [C, R], F16, isOutput=True)
            for f in (0, 1)]

    with tile.TileContext(nc) as tc:
        with (
            tc.tile_pool(name="const", bufs=1) as constp,
            tc.tile_pool(name="feat", bufs=1) as featp,
            tc.tile_pool(name="wpool", bufs=3) as wp,
            tc.tile_pool(name="callp", bufs=3) as callp,
            tc.tile_pool(name="p1s", bufs=3) as p1s,
            tc.tile_pool(name="p2s", bufs=3) as p2s,
            tc.tile_pool(name="dramp", bufs=3, space="DRAM") as dramp,
            tc.tile_pool(name="psump", bufs=1, space="PSUM") as psump,
        ):
            mask = constp.tile([128, 128], F16, tag="mask", name="mask")
            nc.sync.dma_start(out=mask[:], in_=mask_d[:])
            ones = constp.tile([128, 128], F16, tag="ones", name="ones")
            nc.sync.dma_start(out=ones[:], in_=ones_d[:])
            epsln = constp.tile([128, 1], F32, tag="epsln", name="epsln")
            nc.vector.memset(epsln[:], EPS_LN)
            ones1 = constp.tile([128, 1], F16, tag="ones1", name="ones1")
            nc.vector.memset(ones1[:], 1.0)

            x = {}
            for f in (0, 1):
                for ci in (0, 1):
                    t = featp.tile([128, R], F16, tag=f"x{f}{ci}", name=f"x{f}{ci}")
                    nc.sync.dma_start(out=t[:], in_=xin[f][ci * 128:(ci + 1) * 128, :])
                    x[(f, ci)] = t

            def phase1(src, w):
                """K, V token-major from x[src] in paired 128-token blocks;
                partial KV (paired psum) + Ksum (N=1 matmuls); start fp16 AR."""
                npb = len(PAIR_BLOCKS)
                # one PSUM bank per mo: the start=True has_written clear is
                # bank-granular, so the two mo accumulation groups must not
                # share a bank. Ksum lives in the same bank as its KV block
                # (start=False: the bank clear from the KV start leaves its
                # has_written unset, so the first write overwrites stale data).
                kvp = psump.tile([128, 1024], F32, tag="kvp", bufs=1, name="kvp")
                for pb, (t0, n0, n1) in enumerate(PAIR_BLOCKS):
                    nb = n0 + n1
                    pk = psump.tile([128, 512], F32, tag="ring", bufs=5, name="pk")
                    for half, hn in ((0, n0), (1, n1)):
                        if hn == 0:
                            continue
                        hs = t0 + half * 128
                        for ci in (0, 1):
                            nc.tensor.matmul(
                                pk[:hn, half * 256:half * 256 + 256],
                                x[(src, ci)][:, hs:hs + hn],
                                w["wk"][ci][:], start=(ci == 0), stop=(ci == 1))
                    wid = 256 * (2 if n1 else 1)
                    # phi(x) = exp(min(x,0)) + max(x,0), on the paired tile
                    mtmp = p1s.tile([128, 512], F16, tag="mtmp", name="mtmp")
                    nc.vector.tensor_scalar_min(mtmp[:128, :wid], pk[:128, :wid], 0.0)
                    nc.scalar.activation(mtmp[:128, :wid], mtmp[:128, :wid], AF.Exp)
                    ktok = p1s.tile([128, 512], F16, tag="ktok", bufs=3, name="ktok")
                    nc.vector.scalar_tensor_tensor(
                        ktok[:128, :wid], pk[:128, :wid], 0.0, mtmp[:128, :wid],
                        OP.max, OP.add)
                    pv = psump.tile([128, 512], F32, tag="ring", bufs=5, name="pv")
                    for half, hn in ((0, n0), (1, n1)):
                        if hn == 0:
                            continue
                        hs = t0 + half * 128
                        for ci in (0, 1):
                            nc.tensor.matmul(
                                pv[:hn, half * 256:half * 256 + 256],
                                x[(src, ci)][:, hs:hs + hn],
                                w["wv"][ci][:], start=(ci == 0), stop=(ci == 1))
                    vtok = p1s.tile([128, 512], F16, tag="vtok", bufs=3, name="vtok")
                    nc.scalar.copy(vtok[:128, :wid], pv[:128, :wid])
                    first, last = (pb == 0), (pb == npb - 1)
                    for half, hn in ((0, n0), (1, n1)):
                        if hn == 0:
                            if first:  # keep accumulation groups well-formed
                                pass
                            continue
                        for mo in (0, 1):
                            nc.tensor.matmul(
                                kvp[:, 512 * mo:512 * mo + 256],
                                ktok[:hn, half * 256 + mo * 128:half * 256 + (mo + 1) * 128],
                                vtok[:hn, half * 256:half * 256 + 256],
                                start=(first and half == 0),
                                stop=(last and (half == 1 or n1 == 0)))
                            nc.tensor.matmul(
                                kvp[:, 512 * mo + 256:512 * mo + 257],
                                ktok[:hn, half * 256 + mo * 128:half * 256 + (mo + 1) * 128],
                                ones1[:hn, :],
                                start=False,
                                stop=(last and (half == 1 or n1 == 0)),
                                skip_group_check=True)

                arin = dramp.tile([2, 128, 257], F16, tag="arin", name="arin")
                arout = dramp.tile([2, 128, 257], F16, tag="arout", name="arout")
                for mo in (0, 1):
                    # 1/64 here keeps the fp16 wire payload in range; it
                    # cancels exactly against Z (which sees Zden/64).
                    t = callp.tile([128, 257], F16, tag=f"kvsb{mo}", name=f"kvsb{mo}")
                    nc.scalar.activation(t[:], kvp[:, 512 * mo:512 * mo + 257],
                                         AF.Copy, scale=1.0 / 64.0)
                    nc.sync.dma_start(out=arin[mo], in_=t[:])
                nc.gpsimd.collective_compute(
                    "AllReduce", OP.add, replica_groups=GROUPS,
                    ins=[arin.opt()], outs=[arout.opt()])
                return arout

            def finish_kv(arout):
                """Pull the AllReduced KV state back; mask + build Ksum_bcast."""
                kvbd, ksb = [], []
                for ci in (0, 1):
                    t = callp.tile([128, 257], F16, tag=f"kvar{ci}", name=f"kvar{ci}")
                    nc.sync.dma_start(out=t[:], in_=arout[ci])
                    bd = callp.tile([128, 128], F16, tag=f"kvbd{ci}", name=f"kvbd{ci}")
                    nc.vector.tensor_tensor(bd[:], t[:, ci * 128:(ci + 1) * 128],
                                            mask[:], OP.mult)
                    ksf = callp.tile([128, 1], F32, tag=f"ksf{ci}", name=f"ksf{ci}")
                    nc.vector.tensor_copy(ksf[:], t[:, 256:257])
                    kb = callp.tile([128, 128], F16, tag=f"ksb{ci}", name=f"ksb{ci}")
                    nc.vector.tensor_scalar(kb[:], mask[:], ksf[:, 0:1], None,
                                            OP.mult)
                    kvbd.append(bd)
                    ksb.append(kb)
                return kvbd, ksb

            def stage_a(f, w):
                """Q proj + phi(q), all tiles; Exp on a paired [128,960] tile."""
                qphis = []
                for it in range(NTT):
                    ts = slice(it * TT, it * TT + TT)
                    mq = p2s.tile([128, 2 * TT], F16, tag="mq", name="mq")
                    pq_ci = []
                    for ci in (0, 1):
                        p = psump.tile([128, TT], F32, tag="ring", bufs=5, name="pq")
                        for cj in (0, 1):
                            nc.tensor.matmul(p[:],
                                             w["wq"][cj][:, ci * 128:(ci + 1) * 128],
                                             x[(f, cj)][:, ts],
                                             start=(cj == 0), stop=(cj == 1))
                        nc.vector.tensor_scalar_min(
                            mq[:, ci * TT:(ci + 1) * TT], p[:], 0.0)
                        pq_ci.append(p)
                    nc.scalar.activation(mq[:], mq[:], AF.Exp)
                    qp_ci = []
                    for ci in (0, 1):
                        qp = p2s.tile([128, TT], F16, tag=f"qphi{ci}", bufs=10,
                                      name="qp")
                        nc.vector.scalar_tensor_tensor(
                            qp[:], pq_ci[ci][:], 0.0,
                            mq[:, ci * TT:(ci + 1) * TT], OP.max, OP.add)
                        qp_ci.append(qp)
                    qphis.append(qp_ci)
                return qphis

            def stage_b(qphis, kvbd, ksb):
                """Zden -> 1/Z via Exp(-Ln), msgz = (KV q) * Z."""
                msgzs = []
                for it in range(NTT):
                    qphi = qphis[it]
                    zr = []
                    for ci in (0, 1):
                        p = psump.tile([128, TT], F32, tag="ring", bufs=5, name="pz")
                        nc.tensor.matmul(p[:], ksb[ci][:], qphi[ci][:],
                                         start=True, stop=True)
                        lz = p2s.tile([128, TT], F32, tag=f"lz{ci}", name="lz")
                        nc.scalar.activation(lz[:], p[:], AF.Ln)
                        z = p2s.tile([128, TT], F16, tag=f"zr{ci}", name="zr")
                        nc.scalar.activation(z[:], lz[:], AF.Exp, scale=-1.0)
                        zr.append(z)
                    mz_ci = []
                    for ci in (0, 1):
                        p = psump.tile([128, TT], F32, tag="ring", bufs=5, name="pm")
                        nc.tensor.matmul(p[:], kvbd[ci][:], qphi[ci][:],
                                         start=True, stop=True)
                        mz = p2s.tile([128, TT], F16, tag=f"msgz{ci}", bufs=5,
                                      name="mz")
                        nc.vector.tensor_tensor(mz[:], p[:], zr[ci][:], OP.mult)
                        mz_ci.append(mz)
                    msgzs.append(mz_ci)
                return msgzs

            def stage_c(w, msgzs):
                """Centered merge -> yc (PSUM), LN1 stats + scale -> msghat."""
                mhs = []
                for it in range(NTT):
                    msgz = msgzs[it]
                    yc, y2 = [], []
                    for mo in (0, 1):
                        p = psump.tile([128, TT], F32, tag="ring", bufs=5, name="pyc")
                        for ci in (0, 1):
                            nc.tensor.matmul(p[:],
                                             w["wm"][ci][:, mo * 128:(mo + 1) * 128],
                                             msgz[ci][:],
                                             start=(ci == 0), stop=(ci == 1))
                        t2 = p2s.tile([128, TT], F16, tag=f"y2{mo}", name="y2")
                        nc.scalar.activation(t2[:], p[:], AF.Square)
                        yc.append(p)
                        y2.append(t2)
                    pvar = psump.tile([128, TT], F32, tag="ring", bufs=5, name="pvar")
                    for ci in (0, 1):
                        nc.tensor.matmul(pvar[:], ones[:], y2[ci][:],
                                         start=(ci == 0), stop=(ci == 1))
                    rsd = p2s.tile([128, TT], F16, tag="rsd", name="rsd")
                    _act_raw(nc, rsd[:], pvar[:], AF.Rsqrt, bias=epsln[:, 0:1])
                    mh_ci = []
                    for ci in (0, 1):
                        t = p2s.tile([128, TT], F16, tag=f"mh{ci}", bufs=5,
                                     name="mh")
                        nc.vector.tensor_tensor(t[:], yc[ci][:], rsd[:], OP.mult)
                        mh_ci.append(t)
                    mhs.append(mh_ci)
                return mhs

            def stage_d(f, w, mhs):
                """W1 + bias + relu (2 on ACT, 2 on DVE)."""
                r1s = []
                for it in range(NTT):
                    ts = slice(it * TT, it * TT + TT)
                    r1 = []
                    for mo in range(4):
                        p = psump.tile([128, TT], F32, tag="ring", bufs=5, name="pw1")
                        for cj in range(4):
                            rhs = x[(f, cj)][:, ts] if cj < 2 else mhs[it][cj - 2][:]
                            nc.tensor.matmul(p[:],
                                             w["w1"][cj][:, mo * 128:(mo + 1) * 128],
                                             rhs, start=(cj == 0), stop=(cj == 3))
                        t = p2s.tile([128, TT], F16, tag=f"r1{mo}", bufs=5, name="r1")
                        if mo < 2:
                            nc.scalar.activation(t[:], p[:], AF.Relu,
                                                 bias=w["b1"][:, mo:mo + 1])
                        else:
                            nc.vector.tensor_scalar(t[:], p[:],
                                                    w["b1"][:, mo:mo + 1], 0.0,
                                                    OP.add, OP.max)
                        r1.append(t)
                    r1s.append(r1)
                return r1s

            def stage_e(f, w, r1s):
                """Centered W2 -> yd (PSUM), LN2 stats + affine + residual."""
                for it in range(NTT):
                    ts = slice(it * TT, it * TT + TT)
                    r1 = r1s[it]
                    yd, yd2 = [], []
                    for mo in (0, 1):
                        p = psump.tile([128, TT], F32, tag="ring", bufs=5, name="pyd")
                        for cj in range(4):
                            nc.tensor.matmul(p[:],
                                             w["w2"][cj][:, mo * 128:(mo + 1) * 128],
                                             r1[cj][:], start=(cj == 0),
                                             stop=(cj == 3))
                        t2 = p2s.tile([128, TT], F16, tag=f"yd2{mo}", name="yd2")
                        nc.scalar.activation(t2[:], p[:], AF.Square)
                        yd.append(p)
                        yd2.append(t2)
                    pvar2 = psump.tile([128, TT], F32, tag="ring", bufs=5, name="pvar2")
                    for ci in (0, 1):
                        nc.tensor.matmul(pvar2[:], ones[:], yd2[ci][:],
                                         start=(ci == 0), stop=(ci == 1))
                    rsd2 = p2s.tile([128, TT], F16, tag="rsd2", name="rsd2")
                    _act_raw(nc, rsd2[:], pvar2[:], AF.Rsqrt, bias=epsln[:, 0:1])
                    for ci in (0, 1):
                        # x += (yd * ln2_w * rsd2) + ln2_b, as two fused ops
                        t = p2s.tile([128, TT], F16, tag=f"dl{ci}", name="dl")
                        nc.vector.scalar_tensor_tensor(
                            t[:], yd[ci][:], w["l2w"][:, ci:ci + 1], rsd2[:],
                            OP.mult, OP.mult)
                        nc.vector.scalar_tensor_tensor(
                            x[(f, ci)][:, ts], t[:], w["l2b"][:, ci:ci + 1],
                            x[(f, ci)][:, ts], OP.add, OP.add)

            def load_weights(li):
                w = {}
                for nm, dram in (("wq", wq_d), ("wk", wk_d), ("wv", wv_d),
                                 ("wm", wm_d)):
                    tiles = []
                    for ci in (0, 1):
                        t = wp.tile([128, 256], F16, tag=f"{nm}{ci}",
                                    name=f"{nm}{ci}")
                        nc.sync.dma_start(
                            out=t[:], in_=dram[li, ci * 128:(ci + 1) * 128, :])
                        tiles.append(t)
                    w[nm] = tiles
                w["w1"] = []
                for ci in range(4):
                    t = wp.tile([128, 512], F16, tag=f"w1{ci}", name=f"w1{ci}")
                    nc.sync.dma_start(
                        out=t[:], in_=w1_d[li, ci * 128:(ci + 1) * 128, :])
                    w["w1"].append(t)
                w["w2"] = []
                for ci in range(4):
                    t = wp.tile([128, 256], F16, tag=f"w2{ci}", name=f"w2{ci}")
                    nc.sync.dma_start(
                        out=t[:], in_=w2_d[li, ci * 128:(ci + 1) * 128, :])
                    w["w2"].append(t)
                for nm, dram, nf in (("b1", b1_d, 4), ("l2w", l2w_d, 2),
                                     ("l2b", l2b_d, 2)):
                    t = wp.tile([128, nf], F32, tag=nm, name=nm)
                    nc.sync.dma_start(out=t[:], in_=dram[li])
                    w[nm] = t
                return w

            # updates: U_k = (feature, layer). cross layers update f0 with
            # source f1 (pre-update), then f1 with the NEW f0.
            updates = []
            for li in range(n_layers):
                for f in (0, 1):
                    src = f if li % 2 == 0 else 1 - f
                    updates.append((f, li, src))

            wcache = {}

            def get_w(li):
                if li not in wcache:
                    wcache[li] = load_weights(li)
                    # keep only the two most recent layers (wp has bufs=2)
                    for k in list(wcache):
                        if k < li - 1:
                            del wcache[k]
                return wcache[li]

            # phase1 + AR for U_0 up front
            f0, l0, s0 = updates[0]
            ars = {0: phase1(s0, get_w(l0))}

            qphis_pending = {}
            for k, (f, li, src) in enumerate(updates):
                w = get_w(li)
                qphis = qphis_pending.pop(k, None)
                if qphis is None:
                    qphis = stage_a(f, w)
                nxt = updates[k + 1] if k + 1 < len(updates) else None
                if nxt is not None and nxt[1] % 2 == 0:
                    # next update is a self layer: its source feature is
                    # already final -> run its phase1 now, AR hides under B..E
                    ars[k + 1] = phase1(nxt[2], get_w(nxt[1]))
                elif nxt is not None and nxt[1] == li:
                    # first update of a cross layer: the next update's Q/phi
                    # reads the feature THIS update does not write -> hoist
                    # its stage A here so the tensor engine covers our AR wait
                    qphis_pending[k + 1] = stage_a(nxt[0], w)
                kvbd, ksb = finish_kv(ars.pop(k))
                msgzs = stage_b(qphis, kvbd, ksb)
                mhs = stage_c(w, msgzs)
                r1s = stage_d(f, w, mhs)
                stage_e(f, w, r1s)
                if nxt is not None and nxt[1] % 2 == 1:
                    # next is a cross layer: its source was just finalized
                    ars[k + 1] = phase1(nxt[2], get_w(nxt[1]))

            for f in (0, 1):
                for ci in (0, 1):
                    nc.sync.dma_start(out=yout[f][ci * 128:(ci + 1) * 128, :],
                                      in_=x[(f, ci)][:])

    split_multi_waits(nc)
    return nc


def prep_inputs(inputs, n_layers=8):
    """Host-side: shard features, fold ln1 into W1/bias1, center Wm/W2,
    pack constants. Returns in_maps for the 8 cores."""
    f32 = np.float32
    feat0, feat1 = np.asarray(inputs["feat0"]), np.asarray(inputs["feat1"])
    Wq, Wk, Wv, Wm = (np.asarray(inputs[k], dtype=f32)
                      for k in ("Wq", "Wk", "Wv", "Wm"))
    W1, W2 = np.asarray(inputs["W1"], dtype=f32), np.asarray(inputs["W2"], dtype=f32)
    ln1_w, ln1_b = np.asarray(inputs["ln1_w"], dtype=f32), np.asarray(inputs["ln1_b"], dtype=f32)
    ln2_w, ln2_b = np.asarray(inputs["ln2_w"], dtype=f32), np.asarray(inputs["ln2_b"], dtype=f32)

    W1eff = W1[:n_layers].copy()
    W1eff[:, C:, :] *= ln1_w[:n_layers, :, None]
    b1 = np.einsum("lc,lcd->ld", ln1_b[:n_layers], W1[:n_layers, C:, :])
    b1p = np.ascontiguousarray(b1.reshape(n_layers, 4, 128).transpose(0, 2, 1))
    l2wp = np.ascontiguousarray(ln2_w[:n_layers].reshape(n_layers, 2, 128).transpose(0, 2, 1))
    l2bp = np.ascontiguousarray(ln2_b[:n_layers].reshape(n_layers, 2, 128).transpose(0, 2, 1))

    # centered: the merge / W2 matmuls directly emit LN-mean-subtracted rows
    WmC = Wm[:n_layers] - Wm[:n_layers].mean(axis=2, keepdims=True)
    W2C = W2[:n_layers] - W2[:n_layers].mean(axis=2, keepdims=True)

    f16 = np.float16
    idx = np.arange(128)
    # plain head-block mask; the 1/64 range scale is applied on-device
    # before the fp16 AllReduce and cancels exactly in Z.
    blockmask = (idx[:, None] // 32 == idx[None, :] // 32).astype(f16)
    onesC = np.full((128, 128), 1.0 / C, dtype=f16)

    shared = {
        "Wq": np.ascontiguousarray(Wq[:n_layers]).astype(f16),
        "Wk": np.ascontiguousarray(Wk[:n_layers]).astype(f16),
        "Wv": np.ascontiguousarray(Wv[:n_layers]).astype(f16),
        "Wm": np.ascontiguousarray(WmC).astype(f16),
        "W1": np.ascontiguousarray(W1eff).astype(f16),
        "W2": np.ascontiguousarray(W2C).astype(f16),
        "b1p": b1p, "l2wp": l2wp, "l2bp": l2bp,
        "blockmask": blockmask, "onesC": onesC,
    }
    in_maps = []
    for c in range(N_CORES):
        b, h = c // 2, c % 2
        rows = slice(h * R, (h + 1) * R)
        m = dict(shared)
        m["xT0"] = np.ascontiguousarray(feat0[b, rows].T).astype(f16)
        m["xT1"] = np.ascontiguousarray(feat1[b, rows].T).astype(f16)
        in_maps.append(m)
    return in_maps


def assemble_outputs(results):
    feat0 = np.empty((B, L, C), np.float32)
    feat1 = np.empty((B, L, C), np.float32)
    for c in range(N_CORES):
        b, h = c // 2, c % 2
        rows = slice(h * R, (h + 1) * R)
        feat0[b, rows] = results[c]["yT0"].T.astype(np.float32)
        feat1[b, rows] = results[c]["yT1"].T.astype(np.float32)
    return feat0, feat1


_cache = {}


def get_nc(n_layers=8):
    if n_layers not in _cache:
        _cache[n_layers] = build(n_layers)
    return _cache[n_layers]


def kernel(**inputs):
    from concourse.bass_utils import run_bass_kernel_spmd
    nc = get_nc(8)
    in_maps = prep_inputs(inputs, 8)
    res = run_bass_kernel_spmd(nc, in_maps, list(range(N_CORES)))
    return assemble_outputs(res.results)


# revision 25
# speedup vs baseline: 1.0560x; 1.0560x over previous
"""LoFTR-style LocalFeatureTransformer (linear attention) on 8 Trainium2 cores.

Sharding: core c <-> (batch b = c//2, sequence half h = c%2). Each core holds
channel-major [256, 2400] shards of BOTH features, SBUF-resident across all
8 layers. Linear attention's KV state ([hd, hv] plus a K-sum column) is
partial-summed over the local half-sequence and AllReduced (fp16) across the
2-core pair that shares a batch.

v3 structure (per feature update U_k = one encoder layer applied to one
feature; 16 updates total):
  A: Q proj + phi(q) for all 5 token tiles (independent of the AllReduce)
  [self-layer next: phase1(U_{k+1}) + its AllReduce issued HERE, so the
   collective flies under stages B..E]
  fk: unpack the AllReduced KV state
  B: Zden, reciprocal via Exp(-Ln), msg = KV q / Z      (stage loop, 5 tiles)
  C: centered merge -> LN1 (Square/var/Rsqrt/scale)
  D: W1 + bias + relu
  E: centered W2 -> LN2 -> residual into x
  [cross-layer next: phase1(U_{k+1}) + AllReduce issued here]
Stage loops let tile t+1's matmuls overlap tile t's DVE/ACT work (the v2
per-tile fused chain serialized on the PSUM ring).

Key tricks:
- Wm and W2 are CENTERED on the host (W - rowmean(W)): the merge/W2 matmuls
  directly produce mean-subtracted LayerNorm inputs; variance is mean(y^2).
- ln1's affine is folded into W1/bias1 on the host.
- ACT tables: phi/Z use the natural_log_exp table (1/x = Exp(-Ln(x))),
  LN rsqrt is native Rsqrt (one op; validated by the fp32 reference check),
  Copy/Relu/Square live in EVERY table. ~2 table loads per update.
- phi(x) = elu(x)+1 = exp(min(x,0)) + max(x,0): DVE min, ACT exp, fused DVE
  scalar_tensor_tensor. K-phi processes PAIRED 128-token blocks ([128,512]
  PSUM tiles) to amortize per-op overhead; V copy is paired the same way,
  with Ksum accumulated by N=1 matmuls against a constant ones column.
- Zden per (token, head) is computed replicated across each head's 32
  channels with a block-diagonal column-replicated Ksum matrix (1/64 scale
  keeps fp16 KV in range; cancels exactly in Z).
- reference computes msg = Q (K^T (v/S)) Z * S; the /S and *S cancel.
- All matmuls fp16, fp32 accumulation in PSUM.
"""

import numpy as np

import concourse.bass as bass
import concourse.mybir as mybir
import concourse.tile as tile
import bass_rust

N_CORES = 8
B, L, C = 4, 4800, 256
NHEAD, HDIM = 8, 32
R = L // 2              # tokens per core per feature: 2400
TT = 480                # channel-major token tile (moving operand, <=512)
NTT = R // TT           # 5
# paired 128-token blocks for phase1: (start, n0, n1) with n1 == 0 for the tail
PAIR_BLOCKS = [(i * 256, 128, min(128, R - i * 256 - 128)) for i in range(R // 256)]
if R % 256:
    PAIR_BLOCKS.append((R - R % 256, R % 256, 0))
EPS_LN = 1e-5
GROUPS = [[0, 1], [2, 3], [4, 5], [6, 7]]

F32 = mybir.dt.float32
F16 = mybir.dt.float16

_ws_ctr = [0]


def split_multi_waits(nc, max_waits=1):
    """This walrus build accepts only ONE sync-wait per engine instruction.
    After TileContext exit (waits final), move excess waits onto
    EventSemaphore instructions inserted just before the owner."""
    n_split = 0
    for bb in nc.main_func.blocks:
        new_list = []
        for inst in bb.instructions:
            si = inst.sync_info
            waits = list(si.on_wait) if si is not None else []
            if len(waits) > max_waits:
                keep, extra = waits[:max_waits], waits[max_waits:]
                for w in extra:
                    _ws_ctr[0] += 1
                    ev = mybir.InstEventSemaphore(name=f"I-waitsplit-{_ws_ctr[0]}")
                    ev.engine = inst.engine
                    ev.sync_info = bass_rust.SyncInfo(on_wait=[w], on_update=[])
                    nc.register_instruction(ev)
                    new_list.append(ev)
                inst.sync_info = bass_rust.SyncInfo(
                    on_wait=keep, on_update=list(si.on_update)
                )
                n_split += 1
            new_list.append(inst)
        bb.instructions = new_list
    return n_split


def _act_raw(nc, out, in_, func, bias=0.0, scale=1.0):
    """nc.scalar.activation without the Reciprocal/Rsqrt ValueError guard.
    End-to-end error is validated against the fp32 reference."""
    eng = nc.scalar
    inputs = [eng.lower_ap(in_)]
    if not isinstance(bias, float):
        bias_arg = eng.lower_ap(bias)
    else:
        bias_arg = mybir.ImmediateValue(dtype=mybir.dt.float32, value=bias)
    inputs.append(bias_arg)
    inputs.append(mybir.ImmediateValue(dtype=mybir.dt.float32, value=scale))
    inputs.append(mybir.ImmediateValue(dtype=mybir.dt.float32, value=0.0))
    return eng.add_instruction(
        mybir.InstActivation(
            name=nc.get_next_instruction_name(),
            func=func,
            ins=inputs,
            outs=[eng.lower_ap(out)],
        )
    )


def build(n_layers=8):
    nc = bass.Bass("TRN2", target_bir_lowering=False, debug=False,
                   num_devices=N_CORES)
    AF = mybir.ActivationFunctionType
    OP = mybir.AluOpType

    xin = [nc.declare_dram_parameter(f"xT{f}", [C, R], F16, isOutput=False)
           for f in (0, 1)]
    wq_d = nc.declare_dram_parameter("Wq", [n_layers, C, C], F16, isOutput=False)
    wk_d = nc.declare_dram_parameter("Wk", [n_layers, C, C], F16, isOutput=False)
    wv_d = nc.declare_dram_parameter("Wv", [n_layers, C, C], F16, isOutput=False)
    wm_d = nc.declare_dram_parameter("Wm", [n_layers, C, C], F16, isOutput=False)
    w1_d = nc.declare_dram_parameter("W1", [n_layers, 2 * C, 2 * C], F16, isOutput=False)
    w2_d = nc.declare_dram_parameter("W2", [n_layers, 2 * C, C], F16, isOutput=False)
    b1_d = nc.declare_dram_parameter("b1p", [n_layers, 128, 4], F32, isOutput=False)
    l2w_d = nc.declare_dram_parameter("l2wp", [n_layers, 128, 2], F32, isOutput=False)
    l2b_d = nc.declare_dram_parameter("l2bp", [n_layers, 128, 2], F32, isOutput=False)
    mask_d = nc.declare_dram_parameter("blockmask", [128, 128], F16, isOutput=False)
    ones_d = nc.declare_dram_parameter("onesC", [128, 128], F16, isOutput=False)
    yout = [nc.declare_dram_parameter(f"yT{f}", [C, R], F16, isOutput=True)
            for f in (0, 1)]

    with tile.TileContext(nc) as tc:
        with (
            tc.tile_pool(name="const", bufs=1) as constp,
            tc.tile_pool(name="feat", bufs=1) as featp,
            tc.tile_pool(name="wpool", bufs=2) as wp,
            tc.tile_pool(name="callp", bufs=3) as callp,
            tc.tile_pool(name="p1s", bufs=3) as p1s,
            tc.tile_pool(name="p2s", bufs=3) as p2s,
            tc.tile_pool(name="dramp", bufs=2, space="DRAM") as dramp,
            tc.tile_pool(name="psump", bufs=1, space="PSUM") as psump,
        ):
            mask = constp.tile([128, 128], F16, tag="mask", name="mask")
            nc.sync.dma_start(out=mask[:], in_=mask_d[:])
            ones = constp.tile([128, 128], F16, tag="ones", name="ones")
            nc.sync.dma_start(out=ones[:], in_=ones_d[:])
            epsln = constp.tile([128, 1], F32, tag="epsln", name="epsln")
            nc.vector.memset(epsln[:], EPS_LN)
            ones1 = constp.tile([128, 1], F16, tag="ones1", name="ones1")
            nc.vector.memset(ones1[:], 1.0)

            x = {}
            for f in (0, 1):
                for ci in (0, 1):
                    t = featp.tile([128, R], F16, tag=f"x{f}{ci}", name=f"x{f}{ci}")
                    nc.sync.dma_start(out=t[:], in_=xin[f][ci * 128:(ci + 1) * 128, :])
                    x[(f, ci)] = t

            def phase1(src, w):
                """K, V token-major from x[src] in paired 128-token blocks;
                partial KV (paired psum) + Ksum (N=1 matmuls); start fp16 AR."""
                npb = len(PAIR_BLOCKS)
                # one PSUM bank per mo: the start=True has_written clear is
                # bank-granular, so the two mo accumulation groups must not
                # share a bank. Ksum lives in the same bank as its KV block
                # (start=False: the bank clear from the KV start leaves its
                # has_written unset, so the first write overwrites stale data).
                kvp = psump.tile([128, 1024], F32, tag="kvp", bufs=1, name="kvp")
                for pb, (t0, n0, n1) in enumerate(PAIR_BLOCKS):
                    nb = n0 + n1
                    pk = psump.tile([128, 512], F32, tag="ring", bufs=5, name="pk")
                    for half, hn in ((0, n0), (1, n1)):
                        if hn == 0:
                            continue
                        hs = t0 + half * 128
                        for ci in (0, 1):
                            nc.tensor.matmul(
                                pk[:hn, half * 256:half * 256 + 256],
                                x[(src, ci)][:, hs:hs + hn],
                                w["wk"][ci][:], start=(ci == 0), stop=(ci == 1))
                    wid = 256 * (2 if n1 else 1)
                    # phi(x) = exp(min(x,0)) + max(x,0), on the paired tile
                    mtmp = p1s.tile([128, 512], F16, tag="mtmp", name="mtmp")
                    nc.vector.tensor_scalar_min(mtmp[:128, :wid], pk[:128, :wid], 0.0)
                    nc.scalar.activation(mtmp[:128, :wid], mtmp[:128, :wid], AF.Exp)
                    ktok = p1s.tile([128, 512], F16, tag="ktok", bufs=3, name="ktok")
                    nc.vector.scalar_tensor_tensor(
                        ktok[:128, :wid], pk[:128, :wid], 0.0, mtmp[:128, :wid],
                        OP.max, OP.add)
                    pv = psump.tile([128, 512], F32, tag="ring", bufs=5, name="pv")
                    for half, hn in ((0, n0), (1, n1)):
                        if hn == 0:
                            continue
                        hs = t0 + half * 128
                        for ci in (0, 1):
                            nc.tensor.matmul(
                                pv[:hn, half * 256:half * 256 + 256],
                                x[(src, ci)][:, hs:hs + hn],
                                w["wv"][ci][:], start=(ci == 0), stop=(ci == 1))
                    vtok = p1s.tile([128, 512], F16, tag="vtok", bufs=3, name="vtok")
                    nc.scalar.copy(vtok[:128, :wid], pv[:128, :wid])
                    first, last = (pb == 0), (pb == npb - 1)
                    for half, hn in ((0, n0), (1, n1)):
                        if hn == 0:
                            if first:  # keep accumulation groups well-formed
                                pass
                            continue
                        for mo in (0, 1):
                            nc.tensor.matmul(
                                kvp[:, 512 * mo:512 * mo + 256],
                                ktok[:hn, half * 256 + mo * 128:half * 256 + (mo + 1) * 128],
                                vtok[:hn, half * 256:half * 256 + 256],
                                start=(first and half == 0),
                                stop=(last and (half == 1 or n1 == 0)))
                            nc.tensor.matmul(
                                kvp[:, 512 * mo + 256:512 * mo + 257],
                                ktok[:hn, half * 256 + mo * 128:half * 256 + (mo + 1) * 128],
                                ones1[:hn, :],
                                start=False,
                                stop=(last and (half == 1 or n1 == 0)),
                                skip_group_check=True)

                arin = dramp.tile([2, 128, 257], F16, tag="arin", name="arin")
                arout = dramp.tile([2, 128, 257], F16, tag="arout", name="arout")
                for mo in (0, 1):
                    # 1/64 here keeps the fp16 wire payload in range; it
                    # cancels exactly against Z (which sees Zden/64).
                    t = callp.tile([128, 257], F16, tag=f"kvsb{mo}", name=f"kvsb{mo}")
                    nc.scalar.activation(t[:], kvp[:, 512 * mo:512 * mo + 257],
                                         AF.Copy, scale=1.0 / 64.0)
                    nc.sync.dma_start(out=arin[mo], in_=t[:])
                nc.gpsimd.collective_compute(
                    "AllReduce", OP.add, replica_groups=GROUPS,
                    ins=[arin.opt()], outs=[arout.opt()])
                return arout

            def finish_kv(arout):
                """Pull the AllReduced KV state back; mask + build Ksum_bcast."""
                kvbd, ksb = [], []
                for ci in (0, 1):
                    t = callp.tile([128, 257], F16, tag=f"kvar{ci}", name=f"kvar{ci}")
                    nc.sync.dma_start(out=t[:], in_=arout[ci])
                    bd = callp.tile([128, 128], F16, tag=f"kvbd{ci}", name=f"kvbd{ci}")
                    nc.vector.tensor_tensor(bd[:], t[:, ci * 128:(ci + 1) * 128],
                                            mask[:], OP.mult)
                    ksf = callp.tile([128, 1], F32, tag=f"ksf{ci}", name=f"ksf{ci}")
                    nc.vector.tensor_copy(ksf[:], t[:, 256:257])
                    kb = callp.tile([128, 128], F16, tag=f"ksb{ci}", name=f"ksb{ci}")
                    nc.vector.tensor_scalar(kb[:], mask[:], ksf[:, 0:1], None,
                                            OP.mult)
                    kvbd.append(bd)
                    ksb.append(kb)
                return kvbd, ksb

            def stage_a(f, w):
                """Q proj + phi(q), all tiles; Exp on a paired [128,960] tile."""
                qphis = []
                for it in range(NTT):
                    ts = slice(it * TT, it * TT + TT)
                    mq = p2s.tile([128, 2 * TT], F16, tag="mq", name="mq")
                    pq_ci = []
                    for ci in (0, 1):
                        p = psump.tile([128, TT], F32, tag="ring", bufs=5, name="pq")
                        for cj in (0, 1):
                            nc.tensor.matmul(p[:],
                                             w["wq"][cj][:, ci * 128:(ci + 1) * 128],
                                             x[(f, cj)][:, ts],
                                             start=(cj == 0), stop=(cj == 1))
                        nc.vector.tensor_scalar_min(
                            mq[:, ci * TT:(ci + 1) * TT], p[:], 0.0)
                        pq_ci.append(p)
                    nc.scalar.activation(mq[:], mq[:], AF.Exp)
                    qp_ci = []
                    for ci in (0, 1):
                        qp = p2s.tile([128, TT], F16, tag=f"qphi{ci}", bufs=10,
                                      name="qp")
                        nc.vector.scalar_tensor_tensor(
                            qp[:], pq_ci[ci][:], 0.0,
                            mq[:, ci * TT:(ci + 1) * TT], OP.max, OP.add)
                        qp_ci.append(qp)
                    qphis.append(qp_ci)
                return qphis

            def stage_b(qphis, kvbd, ksb):
                """Zden -> 1/Z via Exp(-Ln), msgz = (KV q) * Z."""
                msgzs = []
                for it in range(NTT):
                    qphi = qphis[it]
                    zr = []
                    for ci in (0, 1):
                        p = psump.tile([128, TT], F32, tag="ring", bufs=5, name="pz")
                        nc.tensor.matmul(p[:], ksb[ci][:], qphi[ci][:],
                                         start=True, stop=True)
                        lz = p2s.tile([128, TT], F32, tag=f"lz{ci}", name="lz")
                        nc.scalar.activation(lz[:], p[:], AF.Ln)
                        z = p2s.tile([128, TT], F16, tag=f"zr{ci}", name="zr")
                        nc.scalar.activation(z[:], lz[:], AF.Exp, scale=-1.0)
                        zr.append(z)
                    mz_ci = []
                    for ci in (0, 1):
                        p = psump.tile([128, TT], F32, tag="ring", bufs=5, name="pm")
                        nc.tensor.matmul(p[:], kvbd[ci][:], qphi[ci][:],
                                         start=True, stop=True)
                        mz = p2s.tile([128, TT], F16, tag=f"msgz{ci}", bufs=5,
                                      name="mz")
                        nc.vector.tensor_tensor(mz[:], p[:], zr[ci][:], OP.mult)
                        mz_ci.append(mz)
                    msgzs.append(mz_ci)
                return msgzs

            def stage_c(w, msgzs):
                """Centered merge -> yc (PSUM), LN1 stats + scale -> msghat."""
                mhs = []
                for it in range(NTT):
                    msgz = msgzs[it]
                    yc, y2 = [], []
                    for mo in (0, 1):
                        p = psump.tile([128, TT], F32, tag="ring", bufs=5, name="pyc")
                        for ci in (0, 1):
                            nc.tensor.matmul(p[:],
                                             w["wm"][ci][:, mo * 128:(mo + 1) * 128],
                                             msgz[ci][:],
                                             start=(ci == 0), stop=(ci == 1))
                        t2 = p2s.tile([128, TT], F16, tag=f"y2{mo}", name="y2")
                        nc.scalar.activation(t2[:], p[:], AF.Square)
                        yc.append(p)
                        y2.append(t2)
                    pvar = psump.tile([128, TT], F32, tag="ring", bufs=5, name="pvar")
                    for ci in (0, 1):
                        nc.tensor.matmul(pvar[:], ones[:], y2[ci][:],
                                         start=(ci == 0), stop=(ci == 1))
                    rsd = p2s.tile([128, TT], F16, tag="rsd", name="rsd")
                    _act_raw(nc, rsd[:], pvar[:], AF.Rsqrt, bias=epsln[:, 0:1])
                    mh_ci = []
                    for ci in (0, 1):
                        t = p2s.tile([128, TT], F16, tag=f"mh{ci}", bufs=5,
                                     name="mh")
                        nc.vector.tensor_tensor(t[:], yc[ci][:], rsd[:], OP.mult)
                        mh_ci.append(t)
                    mhs.append(mh_ci)
                return mhs

            def stage_d(f, w, mhs):
                """W1 + bias + relu (2 on ACT, 2 on DVE)."""
                r1s = []
                for it in range(NTT):
                    ts = slice(it * TT, it * TT + TT)
                    r1 = []
                    for mo in range(4):
                        p = psump.tile([128, TT], F32, tag="ring", bufs=5, name="pw1")
                        for cj in range(4):
                            rhs = x[(f, cj)][:, ts] if cj < 2 else mhs[it][cj - 2][:]
                            nc.tensor.matmul(p[:],
                                             w["w1"][cj][:, mo * 128:(mo + 1) * 128],
                                             rhs, start=(cj == 0), stop=(cj == 3))
                        t = p2s.tile([128, TT], F16, tag=f"r1{mo}", bufs=5, name="r1")
                        if mo < 2:
                            nc.scalar.activation(t[:], p[:], AF.Relu,
                                                 bias=w["b1"][:, mo:mo + 1])
                        else:
                            nc.vector.tensor_scalar(t[:], p[:],
                                                    w["b1"][:, mo:mo + 1], 0.0,
                                                    OP.add, OP.max)
                        r1.append(t)
                    r1s.append(r1)
                return r1s

            def stage_e(f, w, r1s):
                """Centered W2 -> yd (PSUM), LN2 stats + affine + residual."""
                for it in range(NTT):
                    ts = slice(it * TT, it * TT + TT)
                    r1 = r1s[it]
                    yd, yd2 = [], []
                    for mo in (0, 1):
                        p = psump.tile([128, TT], F32, tag="ring", bufs=5, name="pyd")
                        for cj in range(4):
                            nc.tensor.matmul(p[:],
                                             w["w2"][cj][:, mo * 128:(mo + 1) * 128],
                                             r1[cj][:], start=(cj == 0),
                                             stop=(cj == 3))
                        t2 = p2s.tile([128, TT], F16, tag=f"yd2{mo}", name="yd2")
                        nc.scalar.activation(t2[:], p[:], AF.Square)
                        yd.append(p)
                        yd2.append(t2)
                    pvar2 = psump.tile([128, TT], F32, tag="ring", bufs=5, name="pvar2")
                    for ci in (0, 1):
                        nc.tensor.matmul(pvar2[:], ones[:], yd2[ci][:],
                                         start=(ci == 0), stop=(ci == 1))
                    rsd2 = p2s.tile([128, TT], F16, tag="rsd2", name="rsd2")
                    _act_raw(nc, rsd2[:], pvar2[:], AF.Rsqrt, bias=epsln[:, 0:1])
                    for ci in (0, 1):
                        # x += (yd * ln2_w * rsd2) + ln2_b, as two fused ops
                        t = p2s.tile([128, TT], F16, tag=f"dl{ci}", name="dl")
                        nc.vector.scalar_tensor_tensor(
                            t[:], yd[ci][:], w["l2w"][:, ci:ci + 1], rsd2[:],
                            OP.mult, OP.mult)
                        nc.vector.scalar_tensor_tensor(
                            x[(f, ci)][:, ts], t[:], w["l2b"][:, ci:ci + 1],
                            x[(f, ci)][:, ts], OP.add, OP.add)

            def load_weights(li):
                w = {}
                for nm, dram in (("wq", wq_d), ("wk", wk_d), ("wv", wv_d),
                                 ("wm", wm_d)):
                    tiles = []
                    for ci in (0, 1):
                        t = wp.tile([128, 256], F16, tag=f"{nm}{ci}",
                                    name=f"{nm}{ci}")
                        nc.sync.dma_start(
                            out=t[:], in_=dram[li, ci * 128:(ci + 1) * 128, :])
                        tiles.append(t)
                    w[nm] = tiles
                w["w1"] = []
                for ci in range(4):
                    t = wp.tile([128, 512], F16, tag=f"w1{ci}", name=f"w1{ci}")
                    nc.sync.dma_start(
                        out=t[:], in_=w1_d[li, ci * 128:(ci + 1) * 128, :])
                    w["w1"].append(t)
                w["w2"] = []
                for ci in range(4):
                    t = wp.tile([128, 256], F16, tag=f"w2{ci}", name=f"w2{ci}")
                    nc.sync.dma_start(
                        out=t[:], in_=w2_d[li, ci * 128:(ci + 1) * 128, :])
                    w["w2"].append(t)
                for nm, dram, nf in (("b1", b1_d, 4), ("l2w", l2w_d, 2),
                                     ("l2b", l2b_d, 2)):
                    t = wp.tile([128, nf], F32, tag=nm, name=nm)
                    nc.sync.dma_start(out=t[:], in_=dram[li])
                    w[nm] = t
                return w

            # updates: U_k = (feature, layer). cross layers update f0 with
            # source f1 (pre-update), then f1 with the NEW f0.
            updates = []
            for li in range(n_layers):
                for f in (0, 1):
                    src = f if li % 2 == 0 else 1 - f
                    updates.append((f, li, src))

            wcache = {}

            def get_w(li):
                if li not in wcache:
                    wcache[li] = load_weights(li)
                    # keep only the two most recent layers (wp has bufs=2)
                    for k in list(wcache):
                        if k < li - 1:
                            del wcache[k]
                return wcache[li]

            # phase1 + AR for U_0 up front
            f0, l0, s0 = updates[0]
            ars = {0: phase1(s0, get_w(l0))}

            qphis_pending = {}
            for k, (f, li, src) in enumerate(updates):
                w = get_w(li)
                qphis = qphis_pending.pop(k, None)
                if qphis is None:
                    qphis = stage_a(f, w)
                nxt = updates[k + 1] if k + 1 < len(updates) else None
                if nxt is not None and nxt[1] % 2 == 0:
                    # next update is a self layer: its source feature is
                    # already final -> run its phase1 now, AR hides under B..E
                    ars[k + 1] = phase1(nxt[2], get_w(nxt[1]))
                elif nxt is not None and nxt[1] == li:
                    # first update of a cross layer: the next update's Q/phi
                    # reads the feature THIS update does not write -> hoist
                    # its stage A here so the tensor engine covers our AR wait
                    qphis_pending[k + 1] = stage_a(nxt[0], w)
                kvbd, ksb = finish_kv(ars.pop(k))
                msgzs = stage_b(qphis, kvbd, ksb)
                mhs = stage_c(w, msgzs)
                r1s = stage_d(f, w, mhs)
                stage_e(f, w, r1s)
                if nxt is not None and nxt[1] % 2 == 1:
                    # next is a cross layer: its source was just finalized
                    ars[k + 1] = phase1(nxt[2], get_w(nxt[1]))

            for f in (0, 1):
                for ci in (0, 1):
                    nc.sync.dma_start(out=yout[f][ci * 128:(ci + 1) * 128, :],
                                      in_=x[(f, ci)][:])

    split_multi_waits(nc)
    return nc


def prep_inputs(inputs, n_layers=8):
    """Host-side: shard features, fold ln1 into W1/bias1, center Wm/W2,
    pack constants. Returns in_maps for the 8 cores."""
    f32 = np.float32
    feat0, feat1 = np.asarray(inputs["feat0"]), np.asarray(inputs["feat1"])
    Wq, Wk, Wv, Wm = (np.asarray(inputs[k], dtype=f32)
                      for k in ("Wq", "Wk", "Wv", "Wm"))
    W1, W2 = np.asarray(inputs["W1"], dtype=f32), np.asarray(inputs["W2"], dtype=f32)
    ln1_w, ln1_b = np.asarray(inputs["ln1_w"], dtype=f32), np.asarray(inputs["ln1_b"], dtype=f32)
    ln2_w, ln2_b = np.asarray(inputs["ln2_w"], dtype=f32), np.asarray(inputs["ln2_b"], dtype=f32)

    W1eff = W1[:n_layers].copy()
    W1eff[:, C:, :] *= ln1_w[:n_layers, :, None]
    b1 = np.einsum("lc,lcd->ld", ln1_b[:n_layers], W1[:n_layers, C:, :])
    b1p = np.ascontiguousarray(b1.reshape(n_layers, 4, 128).transpose(0, 2, 1))
    l2wp = np.ascontiguousarray(ln2_w[:n_layers].reshape(n_layers, 2, 128).transpose(0, 2, 1))
    l2bp = np.ascontiguousarray(ln2_b[:n_layers].reshape(n_layers, 2, 128).transpose(0, 2, 1))

    # centered: the merge / W2 matmuls directly emit LN-mean-subtracted rows
    WmC = Wm[:n_layers] - Wm[:n_layers].mean(axis=2, keepdims=True)
    W2C = W2[:n_layers] - W2[:n_layers].mean(axis=2, keepdims=True)

    f16 = np.float16
    idx = np.arange(128)
    # plain head-block mask; the 1/64 range scale is applied on-device
    # before the fp16 AllReduce and cancels exactly in Z.
    blockmask = (idx[:, None] // 32 == idx[None, :] // 32).astype(f16)
    onesC = np.full((128, 128), 1.0 / C, dtype=f16)

    shared = {
        "Wq": np.ascontiguousarray(Wq[:n_layers]).astype(f16),
        "Wk": np.ascontiguousarray(Wk[:n_layers]).astype(f16),
        "Wv": np.ascontiguousarray(Wv[:n_layers]).astype(f16),
        "Wm": np.ascontiguousarray(WmC).astype(f16),
        "W1": np.ascontiguousarray(W1eff).astype(f16),
        "W2": np.ascontiguousarray(W2C).astype(f16),
        "b1p": b1p, "l2wp": l2wp, "l2bp": l2bp,
        "blockmask": blockmask, "onesC": onesC,
    }
    in_maps = []
    for c in range(N_CORES):
        b, h = c // 2, c % 2
        rows = slice(h * R, (h + 1) * R)
        m = dict(shared)
        m["xT0"] = np.ascontiguousarray(feat0[b, rows].T).astype(f16)
        m["xT1"] = np.ascontiguousarray(feat1[b, rows].T).astype(f16)
        in_maps.append(m)
    return in_maps


def assemble_outputs(results):
    feat0 = np.empty((B, L, C), np.float32)
    feat1 = np.empty((B, L, C), np.float32)
    for c in range(N_CORES):
        b, h = c // 2, c % 2
        rows = slice(h * R, (h + 1) * R)
        feat0[b, rows] = results[c]["yT0"].T.astype(np.float32)
        feat1[b, rows] = results[c]["yT1"].T.astype(np.float32)
    return feat0, feat1


_cache = {}


def get_nc(n_layers=8):
    if n_layers not in _cache:
        _cache[n_layers] = build(n_layers)
    return _cache[n_layers]


def kernel(**inputs):
    from concourse.bass_utils import run_bass_kernel_spmd
    nc = get_nc(8)
    in_maps = prep_inputs(inputs, 8)
    res = run_bass_kernel_spmd(nc, in_maps, list(range(N_CORES)))
    return assemble_outputs(res.results)
